# revision 71
# baseline (speedup 1.0000x reference)
"""Trainium2 Bass kernel for nn_MaxTimesPlusOpeningLiftingP4.

Computation (per rotation i of 4):
  ero[u,c,f]  = min_p (x[u+d_p, c] - ke_i[p,c,f]) * inva_i[p,c,f]
  res[u,f]    = sum_c max_p (tk_i[p,c,f] * ero_pad[u+d_p, c, f] + k_i[p,c,f])
with SAME zero padding on both x and ero, 5x5 window (P=25).

Device layout: 120 SBUF partitions = (c=3, f=8, j=5 row-chunks); pixels on
the free dim in padded per-chunk buffers of 30 rows x 132 cols, with the
core's TWO images interleaved element-wise so every 5x5 shift stays a flat
AP offset.

Key idea: the four rotations' erosions use the SAME 25 (a,b) coefficient
pairs, only assigned to different window offsets (rot90 of the tap grid).
So the 25 affine images s_q = x*a_q + b_q are computed ONCE per segment
pair (on ACT, over the full zero-padded buffer, which makes the padding
value b_q exactly as the reference's zero-pad requires) and both segment
rotations' erosions are pure min-accumulations of shifted views of s_q on
DVE (fp16 tensor_tensor, 2x mode).
Stage 2, segment B (rots 0,1, overlapped with stage-1 of rots 2,3): per
tap, the two rotations' affine terms are written into halves of one pair
tile (split ACT/DVE), then a single pair-merged DVE max over
[NPART, 2*FDB] (saves per-op dispatch overhead and semaphore traffic).
Stage 2, segment C (rots 2,3): two INDEPENDENT single-rotation chains
with their own tmp tiles, rot3 lagging rot2 by 5 taps so rot2's c-sum
(TensorE matmul -> PSUM -> ACT/DVE copy -> DMA) overlaps rot3's tail;
the final tap's max is chunk-split so the c-sum matmuls start per chunk.
Input x is row-split over sync/scalar HWDGE rings + gpsimd SWDGE (DMA
queues are start-latency bound, not bandwidth bound), with the sync ring
dispatching its piece first and stage-1 taps ordered so the first affine
needs only buffer rows 0..26.  Halo exchange between row-chunks is done
with stride-1 all-partition DMAs (j=0/j=4 boundary rows resolve to zero
padding by construction) to keep the dispatch trains short.
Sharding: pure data parallel, 2 images per core on 8 cores.
Host does weight rotation prep, x replication/interleave, and output
reassembly (host work is not on the device clock).
"""
import numpy as np

EPS = 1e-7
B, H, W, C = 16, 128, 128, 3
KH, KW, F = 5, 5, 8
P = KH * KW
NJ = 5
ROWS = [26, 26, 26, 26, 24]
CH_START = [0, 26, 52, 78, 104]
RB = 30
WP = 132
NPART = 120           # (c,f,j): partition = (c*8+f)*5 + j
NCORES = 8
BPC = B // NCORES     # images per core (interleaved in the free dim)
IL = BPC              # interleave factor
WPB = WP * IL         # padded row in elements (264)
FDB = 26 * 128 * IL   # free size per op (6656); j=4 rows 24,25 are garbage
USE_FP16 = True
DVE_AFFINE2 = 12      # stage-2 seg B: of 46 non-init affine slots (2 rots x
                      # 23), how many run on DVE tensor_scalar instead of ACT
DVE_AFFINE2_C = 10    # seg C per-rotation split: of 24, how many on DVE

# rotation-i taps: position p in the rotated grid uses shared image
# s[PERMQ[i][p]]; equivalently processing shared image q touches position
# PERMP[i][q] of rotation i.
_G = np.arange(P).reshape(KH, KW)
PERMP = [[int(np.where(np.rot90(_G, k=i).flatten() == q)[0][0])
          for q in range(P)] for i in range(4)]

_CACHE = {}


def _part(c, f, j):
    return (c * 8 + f) * 5 + j


def _make_weights(kernel, timesKernel):
    """[120, 400] f32; col = (rot*25+p)*4 + slot, slot 0=a 1=b 2=tk 3=k."""
    kernel = np.asarray(kernel, np.float32)
    timesKernel = np.asarray(timesKernel, np.float32)
    k_ero = kernel[::-1, ::-1]
    t_ero = timesKernel[::-1, ::-1]
    Wt = np.zeros((NPART, 4 * P * 4), np.float32)
    for i in range(4):
        k_rot = np.rot90(kernel, k=i, axes=(0, 1)).reshape(P, C, F)
        tk_rot = np.rot90(timesKernel, k=i, axes=(0, 1)).reshape(P, C, F)
        ke_rot = np.rot90(k_ero, k=i, axes=(0, 1)).reshape(P, C, F)
        tke_rot = np.rot90(t_ero, k=i, axes=(0, 1)).reshape(P, C, F)
        a = (1.0 / (tke_rot.astype(np.float64) + EPS)).astype(np.float32)
        b = (-ke_rot * a).astype(np.float32)
        for c in range(C):
            for f in range(F):
                pi = (c * 8 + f) * 5 + np.arange(NJ)
                for p in range(P):
                    col = (i * P + p) * 4
                    Wt[pi, col + 0] = a[p, c, f]
                    Wt[pi, col + 1] = b[p, c, f]
                    Wt[pi, col + 2] = tk_rot[p, c, f]
                    Wt[pi, col + 3] = k_rot[p, c, f]
    return Wt


def _make_csum():
    S = np.zeros((NPART, 40), np.float32)
    for c in range(C):
        for f in range(F):
            for j in range(NJ):
                S[_part(c, f, j), f * NJ + j] = 1.0
    return S


def _build_xrep(x):
    """x [BPC,H,W,C] -> [120, RB*WPB] padded chunks, b interleaved inner."""
    xpad = np.zeros((BPC, H + 6, W + 4, C), np.float32)
    xpad[:, 2:2 + H, 2:2 + W, :] = x
    xr = np.zeros((NPART, RB, WP, IL), np.float32)
    for c in range(C):
        for j in range(NJ):
            blk = xpad[:, CH_START[j]:CH_START[j] + RB, :, c]  # [IL,RB,WP]
            blk = np.moveaxis(blk, 0, -1)                      # [RB,WP,IL]
            for f in range(F):
                xr[(c * 8 + f) * 5 + j] = blk
    return xr.reshape(NPART, RB * WPB)


def _build_program():
    import concourse.bass as bass
    import concourse.bacc as bacc
    import concourse.mybir as mybir
    import concourse.tile as tile

    f32 = mybir.dt.float32
    dt_c = mybir.dt.float16 if USE_FP16 else mybir.dt.float32
    Alu = mybir.AluOpType
    Act = mybir.ActivationFunctionType

    nc = bacc.Bacc("TRN2", target_bir_lowering=False, debug=False,
                   num_devices=NCORES)
    x_in = nc.dram_tensor("x8", [NPART, RB * WPB], dt_c, kind="ExternalInput")
    wts_in = nc.dram_tensor("wts", [NPART, 4 * P * 4], f32, kind="ExternalInput")
    cs_in = nc.dram_tensor("csum", [NPART, 40], dt_c, kind="ExternalInput")
    out_d = nc.dram_tensor("out", [4, 40, FDB], dt_c, kind="ExternalOutput")

    def sb_ap(t, part_off, free_off, dims):
        row = t.shape[1]
        ap = [[sp * row, cnt] for sp, cnt in dims[0]]
        ap += [[se, cnt] for se, cnt in dims[1]]
        return bass.AP(tensor=t.tensor,
                       offset=t.offset + part_off * row + free_off, ap=ap)

    NCHUNK = 4            # c-sum PSUM chunking
    CHW = FDB // NCHUNK   # 1664 elems -> 4-bank PSUM tiles

    with tile.TileContext(nc) as tc:
        with (
            tc.tile_pool(name="singles", bufs=1) as singles,
            tc.tile_pool(name="acc2b", bufs=1) as acc2b_pool,
            tc.tile_pool(name="res", bufs=2) as res_pool,
            tc.tile_pool(name="psum", bufs=2, space="PSUM") as psum_pool,
        ):
            wts = singles.tile([NPART, 4 * P * 4], f32, tag="wts")
            cs = singles.tile([NPART, 40], dt_c, tag="cs")
            zeros = singles.tile([NPART, 2 * WPB], dt_c, tag="zeros")
            xr = singles.tile([NPART, RB * WPB], dt_c, tag="xrep", name="xrep")
            eros = [singles.tile([NPART, RB * WPB], dt_c, tag=f"ero{k}",
                                 name=f"ero{k}") for k in range(4)]
            # DMA queues are start-latency bound, not bandwidth bound (each
            # queue streams ~1MB in ~3us over 15 channels).  So: the sync
            # ring dispatches its x piece FIRST (nothing ahead of it in its
            # FIFO), wts/cs ride the scalar ring ahead of its piece, and
            # gpsimd's SWDGE (slow desc-gen startup) gets a small late-rows
            # piece.  The first shared-affine tap (q=(0,4)) needs buffer
            # rows 0..26 only.
            nc.sync.dma_start(out=wts[:], in_=wts_in[:])
            nc.scalar.dma_start(out=xr[:, :14 * WPB], in_=x_in[:, :14 * WPB])
            nc.sync.dma_start(out=xr[:, 14 * WPB:21 * WPB],
                              in_=x_in[:, 14 * WPB:21 * WPB])
            nc.gpsimd.dma_start(out=xr[:, 21 * WPB:27 * WPB],
                                in_=x_in[:, 21 * WPB:27 * WPB])
            nc.sync.dma_start(out=xr[:, 27 * WPB:], in_=x_in[:, 27 * WPB:])
            nc.scalar.dma_start(out=cs[:], in_=cs_in[:])
            nc.vector.memset(zeros[:], 0.0)
            # dummy 1-element activation: pulls the ACT table load off the
            # critical startup chain (runs as soon as wts lands)
            scratch = singles.tile([NPART, 8], f32, tag="scratch")
            nc.scalar.activation(out=scratch[:, 0:1], in_=wts[:, 0:1],
                                 func=Act.Identity, bias=0.0, scale=1.0)

            def wcol(rot, p, slot):
                return (rot * P + p) * 4 + slot

            def shift_view(t, p):
                return sb_ap(t, 0, (p // 5) * WPB + (p % 5) * IL,
                             [[(1, NPART)], [(WPB, 26), (1, 128 * IL)]])

            def interior(t):
                return sb_ap(t, 0, 2 * WPB + 2 * IL,
                             [[(1, NPART)], [(WPB, 26), (1, 128 * IL)]])

            # stage-2 starts with dh=0 terms so its first ops depend only
            # on the garbage-zero DMA, not the halo DMAs
            ORDER2 = ([12, 10, 11, 13, 14] + list(range(5, 10))
                      + list(range(15, 20)) + list(range(0, 5))
                      + list(range(20, 25)))

            # ero pad hygiene: only the regions never written by stage-1
            # interior writes / halo DMAs need zeroing: the 4 pad columns,
            # j=0's top 2 rows (image boundary) and j=4's bottom 2 rows.
            # (the DVE memsets are emitted by pad_memsets() AFTER the first
            # two stage-1 taps so they fill the DVE gap while ACT computes
            # the second shared affine, instead of burning startup idle)
            for ero in eros:
                nc.sync.dma_start(
                    out=sb_ap(ero, 0, 0, [[(5, 24)], [(1, 2 * WPB)]]),
                    in_=sb_ap(zeros, 0, 0, [[(5, 24)], [(1, 2 * WPB)]]))
                nc.sync.dma_start(
                    out=sb_ap(ero, 4, 28 * WPB, [[(5, 24)], [(1, 2 * WPB)]]),
                    in_=sb_ap(zeros, 4, 0, [[(5, 24)], [(1, 2 * WPB)]]))

            def pad_memsets():
                # on GpSimd (idle engine): keeps ~1.8us off DVE's now-solid
                # op stream
                for ero in eros:
                    nc.gpsimd.memset(
                        sb_ap(ero, 0, 0,
                              [[(1, NPART)], [(WPB, RB), (1, 2 * IL)]]), 0.0)
                    nc.gpsimd.memset(
                        sb_ap(ero, 0, 130 * IL,
                              [[(1, NPART)], [(WPB, RB), (1, 2 * IL)]]), 0.0)

            def s1_step(q, rots, s_pool, first=False):
                """One shared-tap step: s_q on ACT (only the row-span this
                pair's two shifts read) + min-accumulate on DVE.  The FIRST
                tap needs no s image at all: the affine fuses directly into
                the erosion init as interior(ero) = a*shift(xr) + b on DVE
                tensor_scalar (the shifted window reads xr's zero padding,
                which correctly yields b there, matching the reference's
                zero-pad semantics)."""
                if first:
                    for i in rots:
                        p = PERMP[i][q]
                        nc.vector.tensor_scalar(
                            out=interior(eros[i]), in0=shift_view(xr, p),
                            scalar1=wts[:, wcol(0, q, 0):wcol(0, q, 0) + 1],
                            scalar2=wts[:, wcol(0, q, 1):wcol(0, q, 1) + 1],
                            op0=Alu.mult, op1=Alu.add)
                    return
                s = s_pool.tile([NPART, RB * WPB], dt_c, tag="sq", name="sq")
                r0 = min(PERMP[i][q] // 5 for i in rots)
                r1 = max(PERMP[i][q] // 5 for i in rots) + 26
                seg = [[(1, NPART)], [(1, (r1 - r0) * WPB)]]
                nc.scalar.activation(
                    out=sb_ap(s, 0, r0 * WPB, seg),
                    in_=sb_ap(xr, 0, r0 * WPB, seg), func=Act.Identity,
                    bias=wts[:, wcol(0, q, 1):wcol(0, q, 1) + 1],
                    scale=wts[:, wcol(0, q, 0):wcol(0, q, 0) + 1])
                for i in rots:
                    p = PERMP[i][q]
                    nc.vector.tensor_tensor(
                        out=interior(eros[i]), in0=shift_view(s, p),
                        in1=interior(eros[i]), op=Alu.min)

            def fixups(ero):
                """pad fixups per rotation (Sync engine DMAs), 4 dmas total.

                A long per-j dma train (~0.8us dispatch each) at the segment
                boundaries stalled downstream engines via the periodic
                cross-engine barriers, so the halo exchange is done with
                single stride-1 partition-window DMAs over ALL partitions:
                  top: parts 1..119 rows 0,1  <- parts 0..118 rows 26,27.
                    For j=1..4 that is the true halo (j-1 rows 26,27); for
                    j=0 chunks the source is the PREVIOUS cf's j=4 rows
                    26,27, which the garbage-zero dma has just zeroed --
                    exactly the zero padding a j=0 top halo needs.
                  bottom: parts 0..118 rows 28,29 <- parts 1..119 rows 2,3.
                    Correct for j=0..3; j=4 chunks get the NEXT cf's image
                    rows, overwritten right after with zeros."""
                # zero j=4 garbage rows (buffer rows 26,27) FIRST
                nc.sync.dma_start(
                    out=sb_ap(ero, 4, 26 * WPB, [[(5, 24)], [(1, 2 * WPB)]]),
                    in_=sb_ap(zeros, 4, 0, [[(5, 24)], [(1, 2 * WPB)]]))
                nc.sync.dma_start(
                    out=sb_ap(ero, 1, 0, [[(1, 119)], [(1, 2 * WPB)]]),
                    in_=sb_ap(ero, 0, 26 * WPB, [[(1, 119)], [(1, 2 * WPB)]]))
                nc.sync.dma_start(
                    out=sb_ap(ero, 0, 28 * WPB, [[(1, 119)], [(1, 2 * WPB)]]),
                    in_=sb_ap(ero, 1, 2 * WPB, [[(1, 119)], [(1, 2 * WPB)]]))
                # re-zero j=4 bottom halo rows 28,29 (image boundary)
                nc.sync.dma_start(
                    out=sb_ap(ero, 4, 28 * WPB, [[(5, 24)], [(1, 2 * WPB)]]),
                    in_=sb_ap(zeros, 4, 0, [[(5, 24)], [(1, 2 * WPB)]]))

            def csum_out(rot, acc2p, half, tail=False, scratch=None):
                for h in range(NCHUNK):
                    ps = psum_pool.tile([40, CHW], f32, tag="ps", name="ps")
                    base = half * FDB + h * CHW
                    for k in range((CHW + 511) // 512):
                        n0 = k * 512
                        n1 = min(CHW, n0 + 512)
                        nc.tensor.matmul(ps[:, n0:n1], cs[:, 0:40],
                                         acc2p[:, base + n0:base + n1],
                                         start=True, stop=True)
                    if scratch is not None and h >= 2:
                        # tail rotation: stage the last chunks in the dead
                        # acc2B tile so they skip the res-pool recycle wait
                        # (res buf h-2 is still draining its output DMA)
                        res = scratch[0:40, (h - 2) * CHW:(h - 1) * CHW]
                    else:
                        res = res_pool.tile([40, CHW], dt_c, tag="res",
                                            name="res")
                    # tail rotation: alternate PSUM->SBUF copies over ACT and
                    # DVE so they run in parallel (nothing else is live), and
                    # alternate the chunk DMAs over both rings.
                    rv = res if (scratch is not None and h >= 2) else res[:]
                    if tail and h % 2 == 1:
                        nc.vector.tensor_scalar(
                            out=rv, in0=ps[:], scalar1=1.0, scalar2=0.0,
                            op0=Alu.mult, op1=Alu.add)
                    else:
                        nc.scalar.copy(rv, ps[:])
                    eng = nc.scalar if (tail and h % 2 == 1) else nc.sync
                    eng.dma_start(out=out_d[rot, :, h * CHW:(h + 1) * CHW],
                                  in_=rv)

            acc2ps = [None, None]

            def stage2_pair_gen(pair_idx, rots, n_dve_aff, pool, tmp_pool):
                """Generator emitting pair-merged stage-2, one tap per next().

                Per tap: the two rotations' affine terms go to the two halves
                of a pair tile (each on ACT or DVE per the n_dve_aff split),
                then ONE pair-merged DVE max over [NPART, 2*FDB]."""
                acc2p = pool.tile([NPART, 2 * FDB], dt_c, tag="acc2",
                                  name="acc2")
                acc2ps[pair_idx] = acc2p
                p0 = ORDER2[0]
                for j, rot in enumerate(rots):
                    nc.vector.tensor_scalar(
                        out=acc2p[:, j * FDB:(j + 1) * FDB],
                        in0=shift_view(eros[rot], p0),
                        scalar1=wts[:, wcol(rot, p0, 2):wcol(rot, p0, 2) + 1],
                        scalar2=wts[:, wcol(rot, p0, 3):wcol(rot, p0, 3) + 1],
                        op0=Alu.mult, op1=Alu.add)
                yield
                for i_p, p in enumerate(ORDER2[1:]):
                    t = tmp_pool.tile([NPART, 2 * FDB], dt_c, tag="tmp",
                                      name="tmp")
                    for j, rot in enumerate(rots):
                        kslot = 2 * i_p + j
                        s_mul = wts[:, wcol(rot, p, 2):wcol(rot, p, 2) + 1]
                        s_add = wts[:, wcol(rot, p, 3):wcol(rot, p, 3) + 1]
                        dst = t[:, j * FDB:(j + 1) * FDB]
                        if (kslot * n_dve_aff) % 46 < n_dve_aff:
                            nc.vector.tensor_scalar(
                                out=dst, in0=shift_view(eros[rot], p),
                                scalar1=s_mul, scalar2=s_add,
                                op0=Alu.mult, op1=Alu.add)
                        else:
                            nc.scalar.activation(
                                out=dst, in_=shift_view(eros[rot], p),
                                func=Act.Identity, bias=s_add, scale=s_mul)
                    nc.vector.tensor_tensor(out=acc2p[:], in0=t[:],
                                            in1=acc2p[:], op=Alu.max)
                    yield

            def stage2_single_gen(rot, half, acc2p, n_dve_aff, pool):
                """Single-rotation stage-2 chain with its OWN tmp tiles from
                `pool` so the two segment-C chains stay fully independent."""
                a0, a1 = half * FDB, (half + 1) * FDB
                p0 = ORDER2[0]
                nc.vector.tensor_scalar(
                    out=acc2p[:, a0:a1], in0=shift_view(eros[rot], p0),
                    scalar1=wts[:, wcol(rot, p0, 2):wcol(rot, p0, 2) + 1],
                    scalar2=wts[:, wcol(rot, p0, 3):wcol(rot, p0, 3) + 1],
                    op0=Alu.mult, op1=Alu.add)
                yield
                for i_p, p in enumerate(ORDER2[1:]):
                    t = pool.tile([NPART, FDB], dt_c, tag="tmpc", name="tmpc")
                    s_mul = wts[:, wcol(rot, p, 2):wcol(rot, p, 2) + 1]
                    s_add = wts[:, wcol(rot, p, 3):wcol(rot, p, 3) + 1]
                    if (i_p * n_dve_aff) % 24 < n_dve_aff:
                        nc.vector.tensor_scalar(
                            out=t[:], in0=shift_view(eros[rot], p),
                            scalar1=s_mul, scalar2=s_add,
                            op0=Alu.mult, op1=Alu.add)
                    else:
                        nc.scalar.activation(
                            out=t[:], in_=shift_view(eros[rot], p),
                            func=Act.Identity, bias=s_add, scale=s_mul)
                    if i_p == 23:
                        # last tap: chunk-split the max so the csum matmuls
                        # (range-tracked deps) start as soon as each CHW
                        # chunk of the accumulator is final
                        for h in range(NCHUNK):
                            c0, c1 = h * CHW, (h + 1) * CHW
                            nc.vector.tensor_tensor(
                                out=acc2p[:, a0 + c0:a0 + c1],
                                in0=t[:, c0:c1],
                                in1=acc2p[:, a0 + c0:a0 + c1], op=Alu.max)
                    else:
                        nc.vector.tensor_tensor(out=acc2p[:, a0:a1],
                                                in0=t[:],
                                                in1=acc2p[:, a0:a1],
                                                op=Alu.max)
                    yield

            # s_pool and the pair-sized tmp pool are scoped to segments A+B;
            # closing them frees SBUF for segment C's pools.
            with (tc.tile_pool(name="sq", bufs=2) as s_pool,
                  tc.tile_pool(name="tmpb", bufs=2) as tmpb_pool):
                # ---- segment A: erosions of rotations 0,1 (shared-tap) ---
                # ordered by ascending max buffer row needed (rot0 reads row
                # y, rot1 reads row 4-x) so the first taps only need the
                # first input-DMA pieces
                ORDER_A = sorted(range(P),
                                 key=lambda q: max(q // 5, 4 - q % 5) * 32
                                 + min(q // 5, 4 - q % 5))
                for qi, q in enumerate(ORDER_A):
                    s1_step(q, (0, 1), s_pool, first=(qi == 0))
                    if qi == 1:
                        pad_memsets()
                fixups(eros[0])
                fixups(eros[1])

                # ---- segment B: dilation of rots 0,1 || erosion of 2,3 ---
                gB = stage2_pair_gen(0, (0, 1), DVE_AFFINE2, acc2b_pool,
                                     tmpb_pool)
                for q in range(P):
                    s1_step(q, (2, 3), s_pool, first=(q == 0))
                    next(gB, None)
                for _ in gB:
                    pass
                fixups(eros[2])
                fixups(eros[3])

            # ---- segment C: dilation of rotations 2,3 (dual chains) ------
            # rot 3 runs LAG taps behind rot 2, each chain with its OWN tmp
            # tiles (4-buf single-size pool) so they stay independent; rot2
            # finishing early lets csum_out(2) overlap rot3's tail.
            LAG = 5
            with (tc.tile_pool(name="acc2c", bufs=1) as acc2c_pool,
                  tc.tile_pool(name="tmpc", bufs=5) as tmpc_pool):
                csum_out(0, acc2ps[0], 0)
                acc2c = acc2c_pool.tile([NPART, 2 * FDB], dt_c, tag="acc2",
                                        name="acc2c")
                acc2ps[1] = acc2c
                g2 = stage2_single_gen(2, 0, acc2c, DVE_AFFINE2_C, tmpc_pool)
                g3 = stage2_single_gen(3, 1, acc2c, DVE_AFFINE2_C, tmpc_pool)
                next(g2, None)
                next(g3, None)
                done2 = False
                for slot in range(24 + LAG):
                    if slot < 24:
                        next(g2, None)
                    elif not done2:
                        done2 = True
                        csum_out(2, acc2ps[1], 0)
                    if slot == 13:
                        csum_out(1, acc2ps[0], 1)
                    if slot >= LAG:
                        next(g3, None)
                if not done2:
                    csum_out(2, acc2ps[1], 0)
                csum_out(3, acc2ps[1], 1, tail=True, scratch=acc2ps[0])
    nc.compile()
    return nc


def _get_program():
    if "nc" not in _CACHE:
        _CACHE["nc"] = _build_program()
    return _CACHE["nc"]


def kernel(x, kernel, timesKernel):
    x = np.ascontiguousarray(np.asarray(x, np.float32))
    Wt = _make_weights(kernel, timesKernel)
    S = _make_csum()

    nc = _get_program()
    from concourse.bass_utils import run_bass_kernel_spmd
    dt_np = np.float16 if USE_FP16 else np.float32
    in_maps = []
    for i in range(NCORES):
        xrh = _build_xrep(x[i * BPC:(i + 1) * BPC]).astype(dt_np)
        in_maps.append({"x8": xrh, "wts": Wt, "csum": S.astype(dt_np)})

    import os
    trace = os.environ.get("BASS_TRACE", "0") == "1"
    r = run_bass_kernel_spmd(nc, in_maps, core_ids=list(range(NCORES)),
                             trace=trace)
    _CACHE["last_results"] = r
    outs = [m["out"] for m in r.results]

    full = np.empty((B, 4, H, W, F), np.float32)
    for i in range(NCORES):
        O = outs[i].astype(np.float32).reshape(4, 40, 26, 128, IL)
        for rot in range(4):
            for f in range(F):
                for j in range(NJ):
                    rws = ROWS[j]
                    for bb in range(BPC):
                        full[i * BPC + bb, rot,
                             CH_START[j]:CH_START[j] + rws, :, f] = \
                            O[rot, f * NJ + j, :rws, :, bb]
    return full


# revision 72
# speedup vs baseline: 1.0010x; 1.0010x over previous
"""Trainium2 Bass kernel for nn_MaxTimesPlusOpeningLiftingP4.

Computation (per rotation i of 4):
  ero[u,c,f]  = min_p (x[u+d_p, c] - ke_i[p,c,f]) * inva_i[p,c,f]
  res[u,f]    = sum_c max_p (tk_i[p,c,f] * ero_pad[u+d_p, c, f] + k_i[p,c,f])
with SAME zero padding on both x and ero, 5x5 window (P=25).

Device layout: 120 SBUF partitions = (c=3, f=8, j=5 row-chunks); pixels on
the free dim in padded per-chunk buffers of 30 rows x 132 cols, with the
core's TWO images interleaved element-wise so every 5x5 shift stays a flat
AP offset.

Key idea: the four rotations' erosions use the SAME 25 (a,b) coefficient
pairs, only assigned to different window offsets (rot90 of the tap grid).
So the 25 affine images s_q = x*a_q + b_q are computed ONCE per segment
pair (on ACT, over the full zero-padded buffer, which makes the padding
value b_q exactly as the reference's zero-pad requires) and both segment
rotations' erosions are pure min-accumulations of shifted views of s_q on
DVE (fp16 tensor_tensor, 2x mode).
Stage 2, segment B (rots 0,1, overlapped with stage-1 of rots 2,3): per
tap, the two rotations' affine terms are written into halves of one pair
tile (split ACT/DVE), then a single pair-merged DVE max over
[NPART, 2*FDB] (saves per-op dispatch overhead and semaphore traffic).
Stage 2, segment C (rots 2,3): two INDEPENDENT single-rotation chains
with their own tmp tiles, rot3 lagging rot2 by 5 taps so rot2's c-sum
(TensorE matmul -> PSUM -> ACT/DVE copy -> DMA) overlaps rot3's tail;
the final tap's max is chunk-split so the c-sum matmuls start per chunk.
Input x is row-split over sync/scalar HWDGE rings + gpsimd SWDGE (DMA
queues are start-latency bound, not bandwidth bound), with the sync ring
dispatching its piece first and stage-1 taps ordered so the first affine
needs only buffer rows 0..26.  Halo exchange between row-chunks is done
with stride-1 all-partition DMAs (j=0/j=4 boundary rows resolve to zero
padding by construction) to keep the dispatch trains short.
Sharding: pure data parallel, 2 images per core on 8 cores.
Host does weight rotation prep, x replication/interleave, and output
reassembly (host work is not on the device clock).
"""
import numpy as np

EPS = 1e-7
B, H, W, C = 16, 128, 128, 3
KH, KW, F = 5, 5, 8
P = KH * KW
NJ = 5
ROWS = [26, 26, 26, 26, 24]
CH_START = [0, 26, 52, 78, 104]
RB = 30
WP = 132
NPART = 120           # (c,f,j): partition = (c*8+f)*5 + j
NCORES = 8
BPC = B // NCORES     # images per core (interleaved in the free dim)
IL = BPC              # interleave factor
WPB = WP * IL         # padded row in elements (264)
FDB = 26 * 128 * IL   # free size per op (6656); j=4 rows 24,25 are garbage
USE_FP16 = True
DVE_AFFINE2 = 12      # stage-2 seg B: of 46 non-init affine slots (2 rots x
                      # 23), how many run on DVE tensor_scalar instead of ACT
DVE_AFFINE2_C = 10    # seg C per-rotation split: of 24, how many on DVE

# rotation-i taps: position p in the rotated grid uses shared image
# s[PERMQ[i][p]]; equivalently processing shared image q touches position
# PERMP[i][q] of rotation i.
_G = np.arange(P).reshape(KH, KW)
PERMP = [[int(np.where(np.rot90(_G, k=i).flatten() == q)[0][0])
          for q in range(P)] for i in range(4)]

_CACHE = {}


def _part(c, f, j):
    return (c * 8 + f) * 5 + j


def _make_weights(kernel, timesKernel):
    """[120, 400] f32; col = (rot*25+p)*4 + slot, slot 0=a 1=b 2=tk 3=k."""
    kernel = np.asarray(kernel, np.float32)
    timesKernel = np.asarray(timesKernel, np.float32)
    k_ero = kernel[::-1, ::-1]
    t_ero = timesKernel[::-1, ::-1]
    Wt = np.zeros((NPART, 4 * P * 4), np.float32)
    for i in range(4):
        k_rot = np.rot90(kernel, k=i, axes=(0, 1)).reshape(P, C, F)
        tk_rot = np.rot90(timesKernel, k=i, axes=(0, 1)).reshape(P, C, F)
        ke_rot = np.rot90(k_ero, k=i, axes=(0, 1)).reshape(P, C, F)
        tke_rot = np.rot90(t_ero, k=i, axes=(0, 1)).reshape(P, C, F)
        a = (1.0 / (tke_rot.astype(np.float64) + EPS)).astype(np.float32)
        b = (-ke_rot * a).astype(np.float32)
        for c in range(C):
            for f in range(F):
                pi = (c * 8 + f) * 5 + np.arange(NJ)
                for p in range(P):
                    col = (i * P + p) * 4
                    Wt[pi, col + 0] = a[p, c, f]
                    Wt[pi, col + 1] = b[p, c, f]
                    Wt[pi, col + 2] = tk_rot[p, c, f]
                    Wt[pi, col + 3] = k_rot[p, c, f]
    return Wt


def _make_csum():
    S = np.zeros((NPART, 40), np.float32)
    for c in range(C):
        for f in range(F):
            for j in range(NJ):
                S[_part(c, f, j), f * NJ + j] = 1.0
    return S


def _build_xrep(x):
    """x [BPC,H,W,C] -> [120, RB*WPB] padded chunks, b interleaved inner."""
    xpad = np.zeros((BPC, H + 6, W + 4, C), np.float32)
    xpad[:, 2:2 + H, 2:2 + W, :] = x
    xr = np.zeros((NPART, RB, WP, IL), np.float32)
    for c in range(C):
        for j in range(NJ):
            blk = xpad[:, CH_START[j]:CH_START[j] + RB, :, c]  # [IL,RB,WP]
            blk = np.moveaxis(blk, 0, -1)                      # [RB,WP,IL]
            for f in range(F):
                xr[(c * 8 + f) * 5 + j] = blk
    return xr.reshape(NPART, RB * WPB)


def _build_program():
    import concourse.bass as bass
    import concourse.bacc as bacc
    import concourse.mybir as mybir
    import concourse.tile as tile

    f32 = mybir.dt.float32
    dt_c = mybir.dt.float16 if USE_FP16 else mybir.dt.float32
    Alu = mybir.AluOpType
    Act = mybir.ActivationFunctionType

    nc = bacc.Bacc("TRN2", target_bir_lowering=False, debug=False,
                   num_devices=NCORES)
    x_in = nc.dram_tensor("x8", [NPART, RB * WPB], dt_c, kind="ExternalInput")
    wts_in = nc.dram_tensor("wts", [NPART, 4 * P * 4], f32, kind="ExternalInput")
    cs_in = nc.dram_tensor("csum", [NPART, 40], dt_c, kind="ExternalInput")
    out_d = nc.dram_tensor("out", [4, 40, FDB], dt_c, kind="ExternalOutput")

    def sb_ap(t, part_off, free_off, dims):
        row = t.shape[1]
        ap = [[sp * row, cnt] for sp, cnt in dims[0]]
        ap += [[se, cnt] for se, cnt in dims[1]]
        return bass.AP(tensor=t.tensor,
                       offset=t.offset + part_off * row + free_off, ap=ap)

    NCHUNK = 4            # c-sum PSUM chunking
    CHW = FDB // NCHUNK   # 1664 elems -> 4-bank PSUM tiles

    with tile.TileContext(nc) as tc:
        with (
            tc.tile_pool(name="singles", bufs=1) as singles,
            tc.tile_pool(name="acc2b", bufs=1) as acc2b_pool,
            tc.tile_pool(name="res", bufs=2) as res_pool,
            tc.tile_pool(name="psum", bufs=2, space="PSUM") as psum_pool,
        ):
            wts = singles.tile([NPART, 4 * P * 4], f32, tag="wts")
            cs = singles.tile([NPART, 40], dt_c, tag="cs")
            zeros = singles.tile([NPART, 2 * WPB], dt_c, tag="zeros")
            xr = singles.tile([NPART, RB * WPB], dt_c, tag="xrep", name="xrep")
            eros = [singles.tile([NPART, RB * WPB], dt_c, tag=f"ero{k}",
                                 name=f"ero{k}") for k in range(4)]
            # DMA queues are start-latency bound, not bandwidth bound (each
            # queue streams ~1MB in ~3us over 15 channels).  So: the sync
            # ring dispatches its x piece FIRST (nothing ahead of it in its
            # FIFO), wts/cs ride the scalar ring ahead of its piece, and
            # gpsimd's SWDGE (slow desc-gen startup) gets a small late-rows
            # piece.  The first shared-affine tap (q=(0,4)) needs buffer
            # rows 0..26 only.
            nc.sync.dma_start(out=wts[:], in_=wts_in[:])
            nc.scalar.dma_start(out=xr[:, :11 * WPB], in_=x_in[:, :11 * WPB])
            nc.sync.dma_start(out=xr[:, 11 * WPB:18 * WPB],
                              in_=x_in[:, 11 * WPB:18 * WPB])
            nc.gpsimd.dma_start(out=xr[:, 18 * WPB:27 * WPB],
                                in_=x_in[:, 18 * WPB:27 * WPB])
            nc.sync.dma_start(out=xr[:, 27 * WPB:], in_=x_in[:, 27 * WPB:])
            nc.scalar.dma_start(out=cs[:], in_=cs_in[:])
            nc.vector.memset(zeros[:], 0.0)
            # dummy 1-element activation: pulls the ACT table load off the
            # critical startup chain (runs as soon as wts lands)
            scratch = singles.tile([NPART, 8], f32, tag="scratch")
            nc.scalar.activation(out=scratch[:, 0:1], in_=wts[:, 0:1],
                                 func=Act.Identity, bias=0.0, scale=1.0)

            def wcol(rot, p, slot):
                return (rot * P + p) * 4 + slot

            def shift_view(t, p):
                return sb_ap(t, 0, (p // 5) * WPB + (p % 5) * IL,
                             [[(1, NPART)], [(WPB, 26), (1, 128 * IL)]])

            def interior(t):
                return sb_ap(t, 0, 2 * WPB + 2 * IL,
                             [[(1, NPART)], [(WPB, 26), (1, 128 * IL)]])

            # stage-2 starts with dh=0 terms so its first ops depend only
            # on the garbage-zero DMA, not the halo DMAs
            ORDER2 = ([12, 10, 11, 13, 14] + list(range(5, 10))
                      + list(range(15, 20)) + list(range(0, 5))
                      + list(range(20, 25)))

            # ero pad hygiene: only the regions never written by stage-1
            # interior writes / halo DMAs need zeroing: the 4 pad columns,
            # j=0's top 2 rows (image boundary) and j=4's bottom 2 rows.
            # (the DVE memsets are emitted by pad_memsets() AFTER the first
            # two stage-1 taps so they fill the DVE gap while ACT computes
            # the second shared affine, instead of burning startup idle)
            for ero in eros:
                nc.sync.dma_start(
                    out=sb_ap(ero, 0, 0, [[(5, 24)], [(1, 2 * WPB)]]),
                    in_=sb_ap(zeros, 0, 0, [[(5, 24)], [(1, 2 * WPB)]]))
                nc.sync.dma_start(
                    out=sb_ap(ero, 4, 28 * WPB, [[(5, 24)], [(1, 2 * WPB)]]),
                    in_=sb_ap(zeros, 4, 0, [[(5, 24)], [(1, 2 * WPB)]]))

            def pad_memsets():
                # on GpSimd (idle engine): keeps ~1.8us off DVE's now-solid
                # op stream
                for ero in eros:
                    nc.gpsimd.memset(
                        sb_ap(ero, 0, 0,
                              [[(1, NPART)], [(WPB, RB), (1, 2 * IL)]]), 0.0)
                    nc.gpsimd.memset(
                        sb_ap(ero, 0, 130 * IL,
                              [[(1, NPART)], [(WPB, RB), (1, 2 * IL)]]), 0.0)

            def s1_step(q, rots, s_pool, first=False):
                """One shared-tap step: s_q on ACT (only the row-span this
                pair's two shifts read) + min-accumulate on DVE.  The FIRST
                tap needs no s image at all: the affine fuses directly into
                the erosion init as interior(ero) = a*shift(xr) + b on DVE
                tensor_scalar (the shifted window reads xr's zero padding,
                which correctly yields b there, matching the reference's
                zero-pad semantics)."""
                if first:
                    for i in rots:
                        p = PERMP[i][q]
                        nc.vector.tensor_scalar(
                            out=interior(eros[i]), in0=shift_view(xr, p),
                            scalar1=wts[:, wcol(0, q, 0):wcol(0, q, 0) + 1],
                            scalar2=wts[:, wcol(0, q, 1):wcol(0, q, 1) + 1],
                            op0=Alu.mult, op1=Alu.add)
                    return
                s = s_pool.tile([NPART, RB * WPB], dt_c, tag="sq", name="sq")
                r0 = min(PERMP[i][q] // 5 for i in rots)
                r1 = max(PERMP[i][q] // 5 for i in rots) + 26
                seg = [[(1, NPART)], [(1, (r1 - r0) * WPB)]]
                nc.scalar.activation(
                    out=sb_ap(s, 0, r0 * WPB, seg),
                    in_=sb_ap(xr, 0, r0 * WPB, seg), func=Act.Identity,
                    bias=wts[:, wcol(0, q, 1):wcol(0, q, 1) + 1],
                    scale=wts[:, wcol(0, q, 0):wcol(0, q, 0) + 1])
                for i in rots:
                    p = PERMP[i][q]
                    nc.vector.tensor_tensor(
                        out=interior(eros[i]), in0=shift_view(s, p),
                        in1=interior(eros[i]), op=Alu.min)

            def fixups(ero):
                """pad fixups per rotation (Sync engine DMAs), 4 dmas total.

                A long per-j dma train (~0.8us dispatch each) at the segment
                boundaries stalled downstream engines via the periodic
                cross-engine barriers, so the halo exchange is done with
                single stride-1 partition-window DMAs over ALL partitions:
                  top: parts 1..119 rows 0,1  <- parts 0..118 rows 26,27.
                    For j=1..4 that is the true halo (j-1 rows 26,27); for
                    j=0 chunks the source is the PREVIOUS cf's j=4 rows
                    26,27, which the garbage-zero dma has just zeroed --
                    exactly the zero padding a j=0 top halo needs.
                  bottom: parts 0..118 rows 28,29 <- parts 1..119 rows 2,3.
                    Correct for j=0..3; j=4 chunks get the NEXT cf's image
                    rows, overwritten right after with zeros."""
                # zero j=4 garbage rows (buffer rows 26,27) FIRST
                nc.sync.dma_start(
                    out=sb_ap(ero, 4, 26 * WPB, [[(5, 24)], [(1, 2 * WPB)]]),
                    in_=sb_ap(zeros, 4, 0, [[(5, 24)], [(1, 2 * WPB)]]))
                nc.sync.dma_start(
                    out=sb_ap(ero, 1, 0, [[(1, 119)], [(1, 2 * WPB)]]),
                    in_=sb_ap(ero, 0, 26 * WPB, [[(1, 119)], [(1, 2 * WPB)]]))
                nc.sync.dma_start(
                    out=sb_ap(ero, 0, 28 * WPB, [[(1, 119)], [(1, 2 * WPB)]]),
                    in_=sb_ap(ero, 1, 2 * WPB, [[(1, 119)], [(1, 2 * WPB)]]))
                # re-zero j=4 bottom halo rows 28,29 (image boundary)
                nc.sync.dma_start(
                    out=sb_ap(ero, 4, 28 * WPB, [[(5, 24)], [(1, 2 * WPB)]]),
                    in_=sb_ap(zeros, 4, 0, [[(5, 24)], [(1, 2 * WPB)]]))

            def csum_out(rot, acc2p, half, tail=False, scratch=None):
                for h in range(NCHUNK):
                    ps = psum_pool.tile([40, CHW], f32, tag="ps", name="ps")
                    base = half * FDB + h * CHW
                    for k in range((CHW + 511) // 512):
                        n0 = k * 512
                        n1 = min(CHW, n0 + 512)
                        nc.tensor.matmul(ps[:, n0:n1], cs[:, 0:40],
                                         acc2p[:, base + n0:base + n1],
                                         start=True, stop=True)
                    if scratch is not None and h >= 2:
                        # tail rotation: stage the last chunks in the dead
                        # acc2B tile so they skip the res-pool recycle wait
                        # (res buf h-2 is still draining its output DMA)
                        res = scratch[0:40, (h - 2) * CHW:(h - 1) * CHW]
                    else:
                        res = res_pool.tile([40, CHW], dt_c, tag="res",
                                            name="res")
                    # tail rotation: alternate PSUM->SBUF copies over ACT and
                    # DVE so they run in parallel (nothing else is live), and
                    # alternate the chunk DMAs over both rings.
                    rv = res if (scratch is not None and h >= 2) else res[:]
                    if tail and h % 2 == 1:
                        nc.vector.tensor_scalar(
                            out=rv, in0=ps[:], scalar1=1.0, scalar2=0.0,
                            op0=Alu.mult, op1=Alu.add)
                    else:
                        nc.scalar.copy(rv, ps[:])
                    eng = nc.scalar if (tail and h % 2 == 1) else nc.sync
                    eng.dma_start(out=out_d[rot, :, h * CHW:(h + 1) * CHW],
                                  in_=rv)

            acc2ps = [None, None]

            def stage2_pair_gen(pair_idx, rots, n_dve_aff, pool, tmp_pool):
                """Generator emitting pair-merged stage-2, one tap per next().

                Per tap: the two rotations' affine terms go to the two halves
                of a pair tile (each on ACT or DVE per the n_dve_aff split),
                then ONE pair-merged DVE max over [NPART, 2*FDB]."""
                acc2p = pool.tile([NPART, 2 * FDB], dt_c, tag="acc2",
                                  name="acc2")
                acc2ps[pair_idx] = acc2p
                p0 = ORDER2[0]
                for j, rot in enumerate(rots):
                    nc.vector.tensor_scalar(
                        out=acc2p[:, j * FDB:(j + 1) * FDB],
                        in0=shift_view(eros[rot], p0),
                        scalar1=wts[:, wcol(rot, p0, 2):wcol(rot, p0, 2) + 1],
                        scalar2=wts[:, wcol(rot, p0, 3):wcol(rot, p0, 3) + 1],
                        op0=Alu.mult, op1=Alu.add)
                yield
                for i_p, p in enumerate(ORDER2[1:]):
                    t = tmp_pool.tile([NPART, 2 * FDB], dt_c, tag="tmp",
                                      name="tmp")
                    for j, rot in enumerate(rots):
                        kslot = 2 * i_p + j
                        s_mul = wts[:, wcol(rot, p, 2):wcol(rot, p, 2) + 1]
                        s_add = wts[:, wcol(rot, p, 3):wcol(rot, p, 3) + 1]
                        dst = t[:, j * FDB:(j + 1) * FDB]
                        if (kslot * n_dve_aff) % 46 < n_dve_aff:
                            nc.vector.tensor_scalar(
                                out=dst, in0=shift_view(eros[rot], p),
                                scalar1=s_mul, scalar2=s_add,
                                op0=Alu.mult, op1=Alu.add)
                        else:
                            nc.scalar.activation(
                                out=dst, in_=shift_view(eros[rot], p),
                                func=Act.Identity, bias=s_add, scale=s_mul)
                    nc.vector.tensor_tensor(out=acc2p[:], in0=t[:],
                                            in1=acc2p[:], op=Alu.max)
                    yield

            def stage2_single_gen(rot, half, acc2p, n_dve_aff, pool):
                """Single-rotation stage-2 chain with its OWN tmp tiles from
                `pool` so the two segment-C chains stay fully independent."""
                a0, a1 = half * FDB, (half + 1) * FDB
                p0 = ORDER2[0]
                nc.vector.tensor_scalar(
                    out=acc2p[:, a0:a1], in0=shift_view(eros[rot], p0),
                    scalar1=wts[:, wcol(rot, p0, 2):wcol(rot, p0, 2) + 1],
                    scalar2=wts[:, wcol(rot, p0, 3):wcol(rot, p0, 3) + 1],
                    op0=Alu.mult, op1=Alu.add)
                yield
                for i_p, p in enumerate(ORDER2[1:]):
                    t = pool.tile([NPART, FDB], dt_c, tag="tmpc", name="tmpc")
                    s_mul = wts[:, wcol(rot, p, 2):wcol(rot, p, 2) + 1]
                    s_add = wts[:, wcol(rot, p, 3):wcol(rot, p, 3) + 1]
                    if (i_p * n_dve_aff) % 24 < n_dve_aff:
                        nc.vector.tensor_scalar(
                            out=t[:], in0=shift_view(eros[rot], p),
                            scalar1=s_mul, scalar2=s_add,
                            op0=Alu.mult, op1=Alu.add)
                    else:
                        nc.scalar.activation(
                            out=t[:], in_=shift_view(eros[rot], p),
                            func=Act.Identity, bias=s_add, scale=s_mul)
                    if i_p == 23:
                        # last tap: chunk-split the max so the csum matmuls
                        # (range-tracked deps) start as soon as each CHW
                        # chunk of the accumulator is final
                        for h in range(NCHUNK):
                            c0, c1 = h * CHW, (h + 1) * CHW
                            nc.vector.tensor_tensor(
                                out=acc2p[:, a0 + c0:a0 + c1],
                                in0=t[:, c0:c1],
                                in1=acc2p[:, a0 + c0:a0 + c1], op=Alu.max)
                    else:
                        nc.vector.tensor_tensor(out=acc2p[:, a0:a1],
                                                in0=t[:],
                                                in1=acc2p[:, a0:a1],
                                                op=Alu.max)
                    yield

            # s_pool and the pair-sized tmp pool are scoped to segments A+B;
            # closing them frees SBUF for segment C's pools.
            with (tc.tile_pool(name="sq", bufs=2) as s_pool,
                  tc.tile_pool(name="tmpb", bufs=2) as tmpb_pool):
                # ---- segment A: erosions of rotations 0,1 (shared-tap) ---
                # ordered by ascending max buffer row needed (rot0 reads row
                # y, rot1 reads row 4-x) so the first taps only need the
                # first input-DMA pieces
                ORDER_A = sorted(range(P),
                                 key=lambda q: max(q // 5, 4 - q % 5) * 32
                                 + min(q // 5, 4 - q % 5))
                for qi, q in enumerate(ORDER_A):
                    s1_step(q, (0, 1), s_pool, first=(qi == 0))
                    if qi == 1:
                        pad_memsets()
                fixups(eros[0])
                fixups(eros[1])

                # ---- segment B: dilation of rots 0,1 || erosion of 2,3 ---
                gB = stage2_pair_gen(0, (0, 1), DVE_AFFINE2, acc2b_pool,
                                     tmpb_pool)
                for q in range(P):
                    s1_step(q, (2, 3), s_pool, first=(q == 0))
                    next(gB, None)
                for _ in gB:
                    pass
                fixups(eros[2])
                fixups(eros[3])

            # ---- segment C: dilation of rotations 2,3 (dual chains) ------
            # rot 3 runs LAG taps behind rot 2, each chain with its OWN tmp
            # tiles (4-buf single-size pool) so they stay independent; rot2
            # finishing early lets csum_out(2) overlap rot3's tail.
            LAG = 5
            with (tc.tile_pool(name="acc2c", bufs=1) as acc2c_pool,
                  tc.tile_pool(name="tmpc", bufs=5) as tmpc_pool):
                csum_out(0, acc2ps[0], 0)
                acc2c = acc2c_pool.tile([NPART, 2 * FDB], dt_c, tag="acc2",
                                        name="acc2c")
                acc2ps[1] = acc2c
                g2 = stage2_single_gen(2, 0, acc2c, DVE_AFFINE2_C, tmpc_pool)
                g3 = stage2_single_gen(3, 1, acc2c, DVE_AFFINE2_C, tmpc_pool)
                next(g2, None)
                next(g3, None)
                done2 = False
                for slot in range(24 + LAG):
                    if slot < 24:
                        next(g2, None)
                    elif not done2:
                        done2 = True
                        csum_out(2, acc2ps[1], 0)
                    if slot == 13:
                        csum_out(1, acc2ps[0], 1)
                    if slot >= LAG:
                        next(g3, None)
                if not done2:
                    csum_out(2, acc2ps[1], 0)
                csum_out(3, acc2ps[1], 1, tail=True, scratch=acc2ps[0])
    nc.compile()
    return nc


def _get_program():
    if "nc" not in _CACHE:
        _CACHE["nc"] = _build_program()
    return _CACHE["nc"]


def kernel(x, kernel, timesKernel):
    x = np.ascontiguousarray(np.asarray(x, np.float32))
    Wt = _make_weights(kernel, timesKernel)
    S = _make_csum()

    nc = _get_program()
    from concourse.bass_utils import run_bass_kernel_spmd
    dt_np = np.float16 if USE_FP16 else np.float32
    in_maps = []
    for i in range(NCORES):
        xrh = _build_xrep(x[i * BPC:(i + 1) * BPC]).astype(dt_np)
        in_maps.append({"x8": xrh, "wts": Wt, "csum": S.astype(dt_np)})

    import os
    trace = os.environ.get("BASS_TRACE", "0") == "1"
    r = run_bass_kernel_spmd(nc, in_maps, core_ids=list(range(NCORES)),
                             trace=trace)
    _CACHE["last_results"] = r
    outs = [m["out"] for m in r.results]

    full = np.empty((B, 4, H, W, F), np.float32)
    for i in range(NCORES):
        O = outs[i].astype(np.float32).reshape(4, 40, 26, 128, IL)
        for rot in range(4):
            for f in range(F):
                for j in range(NJ):
                    rws = ROWS[j]
                    for bb in range(BPC):
                        full[i * BPC + bb, rot,
                             CH_START[j]:CH_START[j] + rws, :, f] = \
                            O[rot, f * NJ + j, :rws, :, bb]
    return full
